# revision 1
# baseline (speedup 1.0000x reference)
"""Trainium2 Bass kernel for nn_MultiHeadAttention_4913442586758.

Math: with D_MODEL=2, H=2, HS=64, HOD=1 the whole module collapses to
rank-2 attention:
    A_h = Wq[h] @ Wk[h].T / sqrt(64)            [2,2]
    M_h = Wv[h] @ Wo[h] @ Wboth[h:h+1]          [2,2]
    S_h = xp @ A_h @ xp.T   (xp = x + pos_enc)  [C,C]
    P_h = tril-masked exp(S_h)   (no max-subtraction needed: |S| < 0.4)
    y   = sum_h (P_h @ (xp @ M_h)) / rowsum(P_h)

Device computes, per (head, batch), scores transposed S^T[key, query] via
K=6 fp16 hi/lo-compensated matmuls (exact to ~2^-21), exp on ScalarE
(PSUM->SBUF, fp16 out), causal masking as fp16 0/1 multiplies on VectorE,
then PV as [keys,4] x [keys,512] matmuls accumulating [z0,z1,sum,sum]
rows in PSUM, and the softmax division via reciprocal_approx_fast.

Sharding: batch-parallel, 2 batches per core x 8 cores; both heads of a
batch stay on the same core (the head sum happens on-device).
"""

import numpy as np

B, C, H, HS = 16, 2048, 2, 64
NCORES = 8
BPC = B // NCORES          # batches per core
QB = 512                   # query block (free dim of S^T matmuls)
KB = 128                   # key block (partition dim of S^T)
NJ = C // QB               # 4 query blocks
NKC = C // KB              # 16 key chunks
WAVE = 2                   # S banks per exp wave

_cache = {}


def _hilo(v):
    """fp16 hi/lo split: v ~= hi + lo with ~21-bit combined mantissa."""
    hi = v.astype(np.float16)
    lo = (v - hi.astype(np.float64)).astype(np.float16)
    return hi, lo


def _build_program():
    import concourse.bacc as bacc
    import concourse.mybir as mybir
    import concourse.tile as tile

    F32 = mybir.dt.float32
    F16 = mybir.dt.float16
    EXP = mybir.ActivationFunctionType.Exp
    MULT = mybir.AluOpType.mult
    ADD = mybir.AluOpType.add

    nc = bacc.Bacc("TRN2", target_bir_lowering=False, debug=False)

    # per-core inputs (names shared across cores, data differs per core)
    xst_ap = [nc.dram_tensor(f"xst{s}", [KB, C], F16, kind="ExternalInput").ap()
              for s in range(BPC)]
    g_ap = [[nc.dram_tensor(f"g{s}h{h}", [KB, C], F16, kind="ExternalInput").ap()
             for h in range(H)] for s in range(BPC)]
    xn_ap = [[nc.dram_tensor(f"xn{s}h{h}", [KB, 34 * NKC], F16,
                             kind="ExternalInput").ap()
              for h in range(H)] for s in range(BPC)]
    mask_ap = nc.dram_tensor("mask", [KB, 4 * QB], F16, kind="ExternalInput").ap()
    y_ap = [nc.dram_tensor(f"y{s}", [2, C], F32, kind="ExternalOutput").ap()
            for s in range(BPC)]

    with tile.TileContext(nc) as tc:
        import contextlib
        with contextlib.ExitStack() as stack:
            cpool = stack.enter_context(tc.tile_pool(name="consts", bufs=1))
            ppool = stack.enter_context(tc.tile_pool(name="p", bufs=6))
            spool = stack.enter_context(
                tc.tile_pool(name="spsum", bufs=3, space="PSUM"))
            zpool = stack.enter_context(
                tc.tile_pool(name="zpsum", bufs=1, space="PSUM"))
            wpool = stack.enter_context(tc.tile_pool(name="work", bufs=3))

            # load constants; critical-path pieces (stream s=0,h=0, j=0)
            # go first on the sync queue, the rest on the idle gpsimd queue
            xst = [cpool.tile([KB, C], F16, name=f"xst{s}", tag=f"xst{s}")
                   for s in range(BPC)]
            g6 = [[cpool.tile([KB, C], F16, name=f"g{s}{h}", tag=f"g{s}{h}")
                   for h in range(H)] for s in range(BPC)]
            xn = [[cpool.tile([KB, 34 * NKC], F16, name=f"xn{s}{h}",
                              tag=f"xn{s}{h}")
                   for h in range(H)] for s in range(BPC)]
            masks = cpool.tile([KB, 4 * QB], F16, name="masks", tag="masks")
            # dummy exp at t=0 so the ACT table load overlaps the DMA prologue
            warm = wpool.tile([1, 8], F32, name="warm", tag="warm")
            nc.vector.memset(warm[:], 0.0)
            nc.scalar.activation(warm[:], warm[:], EXP)
            nc.sync.dma_start(out=xst[0][:, 0:QB], in_=xst_ap[0][:, 0:QB])
            nc.sync.dma_start(out=g6[0][0][:, 0:QB], in_=g_ap[0][0][:, 0:QB])
            nc.sync.dma_start(out=masks[:], in_=mask_ap[:])
            nc.sync.dma_start(out=xn[0][0][:], in_=xn_ap[0][0][:])
            nc.sync.dma_start(out=g6[0][1][:, 0:QB], in_=g_ap[0][1][:, 0:QB])
            nc.sync.dma_start(out=xn[0][1][:], in_=xn_ap[0][1][:])
            for s in range(BPC):
                for c0 in range(QB, C, QB):
                    for h in range(H):
                        nc.gpsimd.dma_start(out=g6[s][h][:, c0 : c0 + QB],
                                            in_=g_ap[s][h][:, c0 : c0 + QB])
                    nc.gpsimd.dma_start(out=xst[s][:, c0 : c0 + QB],
                                        in_=xst_ap[s][:, c0 : c0 + QB])
                if s != 0:
                    nc.gpsimd.dma_start(out=xst[s][:, 0:QB],
                                        in_=xst_ap[s][:, 0:QB])
                    for h in range(H):
                        nc.gpsimd.dma_start(out=g6[s][h][:, 0:QB],
                                            in_=g_ap[s][h][:, 0:QB])
                        nc.gpsimd.dma_start(out=xn[s][h][:], in_=xn_ap[s][h][:])

            for s in range(BPC):
                for j in (3, 2, 1, 0):
                    u_tiles = []
                    for h in range(H):
                        kcs = list(range(4 * j + 4))
                        z = zpool.tile([34, QB], F32, name=f"z{h}", tag=f"z{h}")
                        for w0 in range(0, len(kcs), WAVE):
                            wave = kcs[w0 : w0 + WAVE]
                            nw = len(wave)
                            S = spool.tile([KB, WAVE * QB], F32, name="S",
                                           tag="S")
                            offs = [max(0, KB * (kc - 4 * j)) for kc in wave]
                            for wi, kc in enumerate(wave):
                                # diagonal chunks: columns < 128i fully masked
                                # -> skip them in both matmul and exp
                                nc.tensor.matmul(
                                    S[:, QB * wi + offs[wi] : QB * (wi + 1)],
                                    g6[s][h][:, KB * kc : KB * (kc + 1)],
                                    xst[s][:, QB * j + offs[wi] : QB * (j + 1)],
                                    start=True, stop=True,
                                )
                            P = ppool.tile([KB, WAVE * QB], F16, name="P",
                                           tag="P")
                            ndiag = sum(1 for o in offs if o == 0)
                            if ndiag:
                                nc.scalar.activation(
                                    P[:, : QB * ndiag], S[:, : QB * ndiag], EXP)
                            for wi in range(ndiag, nw):
                                lo = QB * wi + offs[wi]
                                nc.scalar.activation(
                                    P[:, lo : QB * (wi + 1)],
                                    S[:, lo : QB * (wi + 1)], EXP)
                            for wi, kc in enumerate(wave):
                                psl = P[:, QB * wi : QB * (wi + 1)]
                                if kc >= 4 * j:
                                    i = kc - 4 * j
                                    tri = slice(KB * i, KB * (i + 1))
                                    nc.vector.tensor_tensor(
                                        out=psl[:, tri], in0=psl[:, tri],
                                        in1=masks[:, QB * i + KB * i :
                                                  QB * i + KB * (i + 1)],
                                        op=MULT,
                                    )
                                pvoff = max(0, KB * (kc - 4 * j))
                                nc.tensor.matmul(
                                    z[:, pvoff:],
                                    xn[s][h][:, 34 * kc : 34 * (kc + 1)],
                                    psl[:, pvoff:],
                                    start=(kc == 0), stop=(kc == kcs[-1]),
                                )
                        r = wpool.tile([2, QB], F32, name="r", tag=f"r{h}")
                        nc.vector.reciprocal_approx_fast(out=r[:], in_=z[0:2, :])
                        u = wpool.tile([2, QB], F32, name="u", tag=f"u{h}")
                        nc.vector.tensor_tensor(
                            out=u[:], in0=z[32:34, :], in1=r[:], op=MULT)
                        u_tiles.append(u)
                    y = wpool.tile([2, QB], F32, name="y", tag="y")
                    nc.vector.tensor_tensor(
                        out=y[:], in0=u_tiles[0][:], in1=u_tiles[1][:], op=ADD)
                    nc.sync.dma_start(
                        out=y_ap[s][:, QB * j : QB * (j + 1)], in_=y[:])

    nc.compile()
    return nc


def _prep_inputs(x, Wq, Wk, Wv, Wo, Wboth):
    """Host-side linear input marshaling (all O(B*C))."""
    x = np.asarray(x, np.float64)
    Wq, Wk, Wv, Wo, Wboth = [np.asarray(w, np.float64)
                             for w in (Wq, Wk, Wv, Wo, Wboth)]
    pos = np.arange(C)
    pe = np.stack([np.sin(pos), np.cos(pos)], 1)          # [C,2]
    xp = x + pe[None]                                      # [B,C,2]
    A = np.einsum("hde,hfe->hdf", Wq, Wk) / np.sqrt(HS)    # [H,2,2]
    M = np.stack([Wv[h] @ Wo[h] @ Wboth[h : h + 1] for h in range(H)])

    # causal masks for the 4 diagonal offsets: mask_i[p, c] = c >= 128*i + p
    p_i = np.arange(KB)[:, None]
    c_i = np.arange(QB)[None, :]
    masks = np.concatenate(
        [(c_i >= KB * i + p_i).astype(np.float16) for i in range(NJ)], axis=1)

    in_maps = []
    for core in range(NCORES):
        m = {"mask": masks}
        for s in range(BPC):
            b = core * BPC + s
            xpT = xp[b].T                                  # [2, C]
            xhi, xlo = _hilo(xpT)
            xst6 = np.concatenate([xhi, xlo, xhi], 0)      # [6, C]
            # pad contraction dim to 128: K<128 matmuls stream at half rate
            m[f"xst{s}"] = np.concatenate(
                [xst6, np.zeros((KB - 6, C), np.float16)], 0)
            for h in range(H):
                gg = A[h] @ xpT                            # [2, C]
                ghi, glo = _hilo(gg)
                m[f"g{s}h{h}"] = np.concatenate(
                    [ghi, ghi, glo, np.zeros((KB - 6, C), np.float16)], 0)
                xpM = xp[b] @ M[h]                         # [C, 2]
                # 34 weight cols: [1, 1, zeros(30), xpM0, xpM1] ->
                # z rows 0-1 = sums (part. 0), rows 32-33 = u (part. 32)
                xn = np.zeros((NKC, KB, 34), np.float16)
                xn[:, :, 0:2] = 1.0
                xn[:, :, 32:34] = xpM.reshape(NKC, KB, 2).astype(np.float16)
                m[f"xn{s}h{h}"] = np.ascontiguousarray(
                    xn.transpose(1, 0, 2).reshape(KB, 34 * NKC))
        in_maps.append(m)
    return in_maps


def run(inputs, trace=False):
    from concourse.bass_utils import run_bass_kernel_spmd

    if "nc" not in _cache:
        _cache["nc"] = _build_program()
    nc = _cache["nc"]
    in_maps = _prep_inputs(**inputs)
    res = run_bass_kernel_spmd(
        nc, in_maps, core_ids=list(range(NCORES)), trace=trace)
    y = np.empty((B, C, 2), np.float32)
    for core in range(NCORES):
        for s in range(BPC):
            y[core * BPC + s] = res.results[core][f"y{s}"].T
    return y, res


def kernel(**inputs) -> np.ndarray:
    y, _ = run(inputs, trace=False)
    return y



# revision 23
# speedup vs baseline: 3.4493x; 3.4493x over previous
"""Trainium2 Bass kernel for nn_MultiHeadAttention_4913442586758.

Math: with D_MODEL=2 the scores are rank-2: S = a_q.b_k + c_q.d_k with
|S| <= 0.57, so exp(S) truncated at total degree N=3 is an exact sum of
R=10 rank-1 terms (monomial basis):
    P ~= U V^T,  U[q,r] = a_q^i c_q^j/(i! j!),  V[k,r] = b_k^i d_k^j
(balanced SVD split of the 2x2 score matrix keeps |a|,|b| < 0.8 so all
monomials are <= 1 in magnitude - no cancellation).

Causal-masked softmax over a low-rank P collapses to cumulative sums:
    num_q = sum_r U[q,r] * cumsum_k(V[:,r] * u)[q],   den likewise,
so the device never materializes the C x C matrices: per (batch, head)
it computes block-local cumsums of Vw = V (x) {1, u0, u1} [C, 30] with
16 tril-ones matmuls, chunk prefix offsets via accumulated one-hot +
broadcast-ones matmuls, then one fp16 multiply + segmented reduce on
VectorE against U3, a fast reciprocal, and a TensorE transpose for the
output DMA. Validated end-to-end error ~3e-4 (gate 2e-2).

Sharding: batch-parallel, 2 batches x 2 heads = 4 streams per core.
"""

import math
import numpy as np

B, C, H = 16, 2048, 2
NCORES = 8
BPC = B // NCORES          # batches per core
KB = 128                   # chunk size (partition dim)
NCH = C // KB              # 16 chunks
DEG = 3                    # Taylor degree of exp(S)
EXPS = [(i, n - i) for n in range(DEG + 1) for i in range(n + 1)]
R = len(EXPS)              # 10 monomials
G = 3                      # column groups: {den, num0, num1}
RW = R * G                 # 30 columns per (stream, chunk)
NS = BPC * H               # 4 streams per core
CW = NS * RW               # 120 columns per chunk (all streams)
TOT = NCH * CW             # 1920 columns total

_cache = {}


def _build_program():
    import concourse.bacc as bacc
    import concourse.mybir as mybir
    import concourse.tile as tile

    F32 = mybir.dt.float32
    F16 = mybir.dt.float16
    MULT = mybir.AluOpType.mult
    ADD = mybir.AluOpType.add
    AXX = mybir.AxisListType.X

    nc = bacc.Bacc("TRN2", target_bir_lowering=False, debug=False)

    # consts: [0:128] tril^T, [128:384] one-hot blocks, [384:400] strict
    # chunk-tril, [400:528] identity, [528:2448] row-selectors (15 blocks:
    # block ci-1 is [128,128] with row ci all-ones)
    WCOLS = 528 + 15 * KB
    wts_ap = nc.dram_tensor("wts", [KB, WCOLS], F16, kind="ExternalInput").ap()
    vw_ap = nc.dram_tensor("vw", [KB, TOT], F16, kind="ExternalInput").ap()
    u3_ap = nc.dram_tensor("u3", [KB, TOT], F16, kind="ExternalInput").ap()
    y_ap = [nc.dram_tensor(f"y{s}", [2 * NCH, KB], F16, kind="ExternalOutput").ap()
            for s in range(BPC)]
    import os
    DBG = os.environ.get("KDBG") == "1"
    if DBG:
        cvd_ap = nc.dram_tensor("cvd", [KB, TOT], F32,
                                kind="ExternalOutput").ap()
        totd_ap = nc.dram_tensor("totd", [NCH, CW], F16,
                                 kind="ExternalOutput").ap()

    with tile.TileContext(nc) as tc:
        import contextlib
        with contextlib.ExitStack() as stack:
            cpool = stack.enter_context(tc.tile_pool(name="consts", bufs=1))
            wpool = stack.enter_context(tc.tile_pool(name="work", bufs=1))
            cvp = stack.enter_context(
                tc.tile_pool(name="cvp", bufs=1, space="PSUM"))
            totp = stack.enter_context(
                tc.tile_pool(name="totp", bufs=1, space="PSUM"))
            ytp = stack.enter_context(
                tc.tile_pool(name="ytp", bufs=1, space="PSUM"))

            wts = cpool.tile([KB, WCOLS], F16, name="wts", tag="wts")
            vw = cpool.tile([KB, TOT], F16, name="vw", tag="vw")
            u3 = cpool.tile([KB, TOT], F16, name="u3", tag="u3")

            NP = 4                      # DMA pieces (4 chunks each)
            PW = TOT // NP
            nc.sync.dma_start(out=wts[:, 0:528], in_=wts_ap[:, 0:528])
            for g in range(NP):
                nc.sync.dma_start(out=vw[:, g * PW:(g + 1) * PW],
                                  in_=vw_ap[:, g * PW:(g + 1) * PW])
            nc.gpsimd.dma_start(out=wts[:, 528:WCOLS], in_=wts_ap[:, 528:WCOLS])
            for g in range(NP):
                nc.gpsimd.dma_start(out=u3[:, g * PW:(g + 1) * PW],
                                    in_=u3_ap[:, g * PW:(g + 1) * PW])

            tril = wts[:, 0:128]
            strictT = wts[:, 384:400]
            ident = wts[:, 400:528]

            # chunk totals: totals[m, (s,r)] = sum_k Vw[k, ci=m, s, r]
            totals = totp.tile([NCH, CW], F32, name="totals", tag="totals")
            for ci in range(NCH):
                nc.tensor.matmul(
                    totals[:],
                    wts[:, 128 + 16 * ci: 128 + 16 * (ci + 1)],
                    vw[:, ci * CW:(ci + 1) * CW],
                    start=(ci == 0), stop=(ci == NCH - 1),
                )
            # zero-padded totals (K=128 contraction for the prefix matmul)
            tots = wpool.tile([KB, CW], F16, name="tots", tag="tots")
            nc.vector.memset(tots[:], 0.0)
            nc.vector.tensor_copy(tots[0:NCH, :], totals[:])
            # prefix offsets: off[ci] = sum_{cj<ci} totals[cj]
            offp = totp.tile([NCH, CW], F32, name="offp", tag="offp")
            nc.tensor.matmul(offp[:], strictT, tots[:], start=True, stop=True)
            offs = wpool.tile([KB, CW], F16, name="offs", tag="offs")
            nc.vector.memset(offs[:], 0.0)
            nc.vector.tensor_copy(offs[0:NCH, :], offp[:])

            # block-local cumsums plus broadcast prefix offset, emitted as
            # adjacent accumulation pairs per chunk:
            # cv[q, ci, s, r] = sum_{k<=q} Vw[k, ci, s, r] + off[ci, s, r]
            # chunk slots padded to 128 fp32 so no slot crosses a PSUM bank
            CP = 128
            cv = cvp.tile([KB, NCH * CP], F32, name="cv", tag="cv")
            nc.tensor.matmul(cv[:, 0:CW], tril, vw[:, 0:CW],
                             start=True, stop=True)
            for ci in range(1, NCH):
                nc.tensor.matmul(
                    cv[:, ci * CP:ci * CP + CW], tril,
                    vw[:, ci * CW:(ci + 1) * CW],
                    start=True, stop=False,
                )
                nc.tensor.matmul(
                    cv[:, ci * CP:ci * CP + CW],
                    wts[:, 528 + KB * (ci - 1):528 + KB * ci], offs[:],
                    start=False, stop=True,
                )

            cvv = cv.rearrange("p (c w) -> p c w", w=CP)
            if DBG:
                nc.sync.dma_start(out=totd_ap[:], in_=tots[0:NCH, :])
                cvd = cpool.tile([KB, TOT], F32, name="cvd", tag="cvd")
                for g in range(NP):
                    nc.scalar.copy(
                        cvd[:, g * PW:(g + 1) * PW].rearrange(
                            "p (c w) -> p c w", w=CW),
                        cvv[:, 4 * g:4 * (g + 1), 0:CW])
                nc.sync.dma_start(out=cvd_ap[:], in_=cvd[:])

            # tmp = U3 * CV ; red[q, (ci,s,g)] = sum_r tmp
            tmp = wpool.tile([KB, TOT], F16, name="tmp", tag="tmp")
            red = wpool.tile([KB, NCH * NS * G], F32, name="red", tag="red")
            NR = NCH * NS * G // NP
            for g in range(NP):
                nc.vector.tensor_tensor(
                    out=tmp[:, g * PW:(g + 1) * PW].rearrange(
                        "p (c w) -> p c w", w=CW),
                    in0=cvv[:, 4 * g:4 * (g + 1), 0:CW],
                    in1=u3[:, g * PW:(g + 1) * PW].rearrange(
                        "p (c w) -> p c w", w=CW), op=MULT)
                nc.vector.tensor_reduce(
                    out=red[:, g * NR:(g + 1) * NR],
                    in_=tmp[:, g * PW:(g + 1) * PW].rearrange(
                        "p (a r) -> p a r", r=R),
                    axis=AXX, op=ADD)

            # per-stream: r = 1/den ; y = num * r ; head-add ; transpose
            redv = red.rearrange("p (c s g) -> p c s g", s=NS, g=G)
            ys = []
            for s in range(NS):
                rs = wpool.tile([KB, NCH], F32, name="rs", tag=f"rs{s}")
                nc.vector.reciprocal_approx_fast(out=rs[:], in_=redv[:, :, s, 0])
                y_s = wpool.tile([KB, NCH, 2], F16, name="ys", tag=f"ys{s}")
                nc.vector.tensor_tensor(
                    out=y_s[:], in0=redv[:, :, s, 1:3],
                    in1=rs[:].unsqueeze(2).broadcast_to((KB, NCH, 2)), op=MULT)
                ys.append(y_s)
            for bl in range(BPC):
                yb = wpool.tile([KB, NCH * 2], F16, name="yb", tag=f"yb{bl}")
                nc.vector.tensor_tensor(
                    out=yb[:], in0=ys[2 * bl][:].rearrange("p a b -> p (a b)"),
                    in1=ys[2 * bl + 1][:].rearrange("p a b -> p (a b)"), op=ADD)
                yt = ytp.tile([NCH * 2, KB], F16, name="yt", tag=f"yt{bl}")
                nc.tensor.transpose(yt[:], yb[:], ident)
                yo = wpool.tile([NCH * 2, KB], F16, name="yo", tag=f"yo{bl}")
                nc.scalar.copy(yo[:], yt[:])
                nc.sync.dma_start(out=y_ap[bl][:], in_=yo[:])

    nc.compile()
    return nc


def _prep_inputs(x, Wq, Wk, Wv, Wo, Wboth):
    """Host-side linear prep: rank-2 factors and monomial bases, O(B*C*R)."""
    x = np.asarray(x, np.float64)
    Wq, Wk, Wv, Wo, Wboth = [np.asarray(w, np.float64)
                             for w in (Wq, Wk, Wv, Wo, Wboth)]
    pos = np.arange(C)
    pe = np.stack([np.sin(pos), np.cos(pos)], 1)           # [C,2]
    xp = x + pe[None]                                       # [B,C,2]
    A = np.einsum("hde,hfe->hdf", Wq, Wk) / np.sqrt(64)     # [H,2,2]
    M = np.stack([Wv[h] @ Wo[h] @ Wboth[h:h + 1] for h in range(H)])

    U3h, Vwh = [], []
    for h in range(H):
        Uh, sh, Vth = np.linalg.svd(A[h])
        a = xp @ (Uh * np.sqrt(sh))                         # [B,C,2]
        b = xp @ (Vth.T * np.sqrt(sh))
        u = xp @ M[h]                                       # [B,C,2]
        U = np.stack([a[..., 0] ** i * a[..., 1] ** j
                      / (math.factorial(i) * math.factorial(j))
                      for (i, j) in EXPS], -1)              # [B,C,R]
        V = np.stack([b[..., 0] ** i * b[..., 1] ** j for (i, j) in EXPS], -1)
        Vw = np.concatenate([V, V * u[..., 0:1], V * u[..., 1:2]], -1)
        U3h.append(np.tile(U, (1, 1, G)))                   # [B,C,RW]
        Vwh.append(Vw)

    # consts
    q_i = np.arange(KB)
    wts = np.zeros((KB, 528 + 15 * KB), np.float16)
    wts[:, 0:128] = (q_i[:, None] <= q_i[None, :])          # tril^T
    for ci in range(NCH):
        wts[:, 128 + 16 * ci + ci] = 1.0                    # one-hot col ci
    wts[0:NCH, 384:400] = (np.arange(NCH)[:, None]
                           < np.arange(NCH)[None, :])       # strict chunk-tril
    wts[:, 400:528] = np.eye(KB)
    for ci in range(1, NCH):                                # row-selectors
        wts[ci, 528 + KB * (ci - 1):528 + KB * ci] = 1.0

    in_maps = []
    for core in range(NCORES):
        vw = np.empty((KB, NCH, NS, RW), np.float16)
        u3 = np.empty((KB, NCH, NS, RW), np.float16)
        for s in range(NS):
            b_ = core * BPC + s // H
            h = s % H
            # [C, RW] -> [NCH, KB, RW] -> [KB, NCH, RW]
            vw[:, :, s] = Vwh[h][b_].reshape(NCH, KB, RW).transpose(1, 0, 2)
            u3[:, :, s] = U3h[h][b_].reshape(NCH, KB, RW).transpose(1, 0, 2)
        in_maps.append({
            "wts": wts,
            "vw": np.ascontiguousarray(vw.reshape(KB, TOT)),
            "u3": np.ascontiguousarray(u3.reshape(KB, TOT)),
        })
    return in_maps


def run(inputs, trace=False):
    from concourse.bass_utils import run_bass_kernel_spmd

    if "nc" not in _cache:
        _cache["nc"] = _build_program()
    nc = _cache["nc"]
    in_maps = _prep_inputs(**inputs)
    res = run_bass_kernel_spmd(
        nc, in_maps, core_ids=list(range(NCORES)), trace=trace)
    y = np.empty((B, C, 2), np.float32)
    for core in range(NCORES):
        for bl in range(BPC):
            yt = res.results[core][f"y{bl}"].astype(np.float32)  # [32,128]
            y[core * BPC + bl] = yt.reshape(NCH, 2, KB).transpose(
                0, 2, 1).reshape(C, 2)
    return y, res


def kernel(**inputs) -> np.ndarray:
    y, _ = run(inputs, trace=False)
    return y


# revision 33
# speedup vs baseline: 3.7355x; 1.0830x over previous
"""Trainium2 Bass kernel for nn_MultiHeadAttention_4913442586758.

Math: with D_MODEL=2 the scores are rank-2: S = a_q.b_k + c_q.d_k with
|S| <= 0.57, so exp(S) truncated at total degree N=3 is an exact sum of
R=10 rank-1 terms (monomial basis):
    P ~= U V^T,  U[q,r] = a_q^i c_q^j/(i! j!),  V[k,r] = b_k^i d_k^j
(balanced SVD split of the 2x2 score matrix keeps |a|,|b| < 0.8 so all
monomials are <= 1 in magnitude - no cancellation).

Causal-masked softmax over a low-rank P collapses to cumulative sums:
    num_q = sum_r U[q,r] * cumsum_k(V[:,r] * u)[q],   den likewise,
so the device never materializes the C x C matrices: per (batch, head)
it computes block-local cumsums of Vw = V (x) {1, u0, u1} [C, 30] with
16 tril-ones matmuls, chunk prefix offsets via accumulated one-hot +
broadcast-ones matmuls, then one fp16 multiply + segmented reduce on
VectorE against U3, a fast reciprocal, and a TensorE transpose for the
output DMA. Validated end-to-end error ~3e-4 (gate 2e-2).

Sharding: batch-parallel, 2 batches x 2 heads = 4 streams per core.
"""

import math
import numpy as np

B, C, H = 16, 2048, 2
NCORES = 8
BPC = B // NCORES          # batches per core
KB = 128                   # chunk size (partition dim)
NCH = C // KB              # 16 chunks
DEG = 3                    # Taylor degree of exp(S)
EXPS = [(i, n - i) for n in range(DEG + 1) for i in range(n + 1)]
R = len(EXPS)              # 10 monomials
G = 3                      # column groups: {den, num0, num1}
RW = R * G                 # 30 columns per (stream, chunk)
NS = BPC * H               # 4 streams per core
CW = NS * RW               # 120 columns per chunk (all streams)
TOT = NCH * CW             # 1920 columns total

_cache = {}


def _build_program():
    import concourse.bacc as bacc
    import concourse.mybir as mybir
    import concourse.tile as tile

    F32 = mybir.dt.float32
    F16 = mybir.dt.float16
    MULT = mybir.AluOpType.mult
    ADD = mybir.AluOpType.add
    AXX = mybir.AxisListType.X

    nc = bacc.Bacc("TRN2", target_bir_lowering=False, debug=False)

    # consts: [0:128] tril^T, [128:384] one-hot blocks, [384:400] strict
    # chunk-tril, [400:528] identity, [528] partition index, [529:544]
    # chunk index row (values 1..15)
    WCOLS = 544
    wts_ap = nc.dram_tensor("wts", [KB, WCOLS], F16, kind="ExternalInput").ap()
    vw_ap = nc.dram_tensor("vw", [KB, TOT], F16, kind="ExternalInput").ap()
    u3_ap = nc.dram_tensor("u3", [KB, TOT], F16, kind="ExternalInput").ap()
    y_ap = [nc.dram_tensor(f"y{s}", [2 * NCH, KB], F16, kind="ExternalOutput").ap()
            for s in range(BPC)]
    import os
    DBG = os.environ.get("KDBG") == "1"
    if DBG:
        cvd_ap = nc.dram_tensor("cvd", [KB, TOT], F32,
                                kind="ExternalOutput").ap()
        totd_ap = nc.dram_tensor("totd", [NCH, CW], F16,
                                 kind="ExternalOutput").ap()

    with tile.TileContext(nc) as tc:
        import contextlib
        with contextlib.ExitStack() as stack:
            cpool = stack.enter_context(tc.tile_pool(name="consts", bufs=1))
            wpool = stack.enter_context(tc.tile_pool(name="work", bufs=1))
            cvp = stack.enter_context(
                tc.tile_pool(name="cvp", bufs=1, space="PSUM"))
            totp = stack.enter_context(
                tc.tile_pool(name="totp", bufs=1, space="PSUM"))
            ytp = stack.enter_context(
                tc.tile_pool(name="ytp", bufs=1, space="PSUM"))

            wts = cpool.tile([KB, WCOLS], F16, name="wts", tag="wts")
            vw = cpool.tile([KB, TOT], F16, name="vw", tag="vw")
            u3 = cpool.tile([KB, TOT], F16, name="u3", tag="u3")

            # PE warm-up: ~3.4us of dummy matmuls releases the HAM clock
            # gate so the real matmuls run at 2.4 GHz instead of 1.2.
            # They scribble on cv's last bank; chunks 12-15 reset it later.
            CP = 128
            cv = cvp.tile([KB, NCH * CP], F32, name="cv", tag="cv")
            dum = cpool.tile([KB, 512], F16, name="dum", tag="dum")
            nc.vector.memset(dum[:], 0.0)
            for _ in range(6):
                nc.tensor.matmul(cv[:, 12 * CP:16 * CP], dum[:, 0:128],
                                 dum[:], start=True, stop=True)

            NP = 4                      # DMA pieces (4 chunks each)
            PW = TOT // NP
            nc.sync.dma_start(out=wts[:], in_=wts_ap[:])
            for g in range(2):
                nc.sync.dma_start(out=vw[:, g * PW:(g + 1) * PW],
                                  in_=vw_ap[:, g * PW:(g + 1) * PW])
            for g in range(2, NP):
                nc.gpsimd.dma_start(out=vw[:, g * PW:(g + 1) * PW],
                                    in_=vw_ap[:, g * PW:(g + 1) * PW])
            for g in range(NP):
                nc.gpsimd.dma_start(out=u3[:, g * PW:(g + 1) * PW],
                                    in_=u3_ap[:, g * PW:(g + 1) * PW])

            tril = wts[:, 0:128]
            strictT = wts[:, 384:400]
            ident = wts[:, 400:528]
            # row-selector blocks rs[:, 128j:128j+128] = [p == j+1], built
            # on device: one is_equal against the partition-index column
            rs = cpool.tile([KB, 15 * KB], F16, name="rs", tag="rs")
            nc.vector.tensor_tensor(
                out=rs[:].rearrange("p (c q) -> p c q", q=KB),
                in0=wts[:, 528:529].unsqueeze(2).broadcast_to((KB, 15, KB)),
                in1=wts[:, 529:544].unsqueeze(2).broadcast_to((KB, 15, KB)),
                op=mybir.AluOpType.is_equal)

            # chunk totals: totals[m, (s,r)] = sum_k Vw[k, ci=m, s, r]
            totals = totp.tile([NCH, CW], F32, name="totals", tag="totals")
            for ci in range(NCH):
                nc.tensor.matmul(
                    totals[:],
                    wts[:, 128 + 16 * ci: 128 + 16 * (ci + 1)],
                    vw[:, ci * CW:(ci + 1) * CW],
                    start=(ci == 0), stop=(ci == NCH - 1),
                )
            # zero-padded totals (K=128 contraction for the prefix matmul)
            tots = wpool.tile([KB, CW], F16, name="tots", tag="tots")
            nc.vector.memset(tots[:], 0.0)
            nc.vector.tensor_copy(tots[0:NCH, :], totals[:])
            # prefix offsets: off[ci] = sum_{cj<ci} totals[cj]
            offp = totp.tile([NCH, CW], F32, name="offp", tag="offp")
            nc.tensor.matmul(offp[:], strictT, tots[:], start=True, stop=True)
            offs = wpool.tile([KB, CW], F16, name="offs", tag="offs")
            nc.vector.memset(offs[:], 0.0)
            nc.vector.tensor_copy(offs[0:NCH, :], offp[:])

            # block-local cumsums plus broadcast prefix offset, emitted as
            # adjacent accumulation pairs per chunk:
            # cv[q, ci, s, r] = sum_{k<=q} Vw[k, ci, s, r] + off[ci, s, r]
            # chunk slots padded to 128 fp32 so no slot crosses a PSUM bank
            nc.tensor.matmul(cv[:, 0:CW], tril, vw[:, 0:CW],
                             start=True, stop=True)
            for ci in range(1, NCH):
                nc.tensor.matmul(
                    cv[:, ci * CP:ci * CP + CW], tril,
                    vw[:, ci * CW:(ci + 1) * CW],
                    start=True, stop=False,
                )
                nc.tensor.matmul(
                    cv[:, ci * CP:ci * CP + CW],
                    rs[:, KB * (ci - 1):KB * ci], offs[:],
                    start=False, stop=True,
                )

            cvv = cv.rearrange("p (c w) -> p c w", w=CP)
            if DBG:
                nc.sync.dma_start(out=totd_ap[:], in_=tots[0:NCH, :])
                cvd = cpool.tile([KB, TOT], F32, name="cvd", tag="cvd")
                for g in range(NP):
                    nc.scalar.copy(
                        cvd[:, g * PW:(g + 1) * PW].rearrange(
                            "p (c w) -> p c w", w=CW),
                        cvv[:, 4 * g:4 * (g + 1), 0:CW])
                nc.sync.dma_start(out=cvd_ap[:], in_=cvd[:])

            # tmp = U3 * CV ; red[q, (ci,s,g)] = sum_r tmp
            tmp = wpool.tile([KB, TOT], F16, name="tmp", tag="tmp")
            red = wpool.tile([KB, NCH * NS * G], F32, name="red", tag="red")
            NR = NCH * NS * G // NP
            for g in range(NP):
                nc.vector.tensor_tensor(
                    out=tmp[:, g * PW:(g + 1) * PW].rearrange(
                        "p (c w) -> p c w", w=CW),
                    in0=cvv[:, 4 * g:4 * (g + 1), 0:CW],
                    in1=u3[:, g * PW:(g + 1) * PW].rearrange(
                        "p (c w) -> p c w", w=CW), op=MULT)
                nc.vector.tensor_reduce(
                    out=red[:, g * NR:(g + 1) * NR],
                    in_=tmp[:, g * PW:(g + 1) * PW].rearrange(
                        "p (a r) -> p a r", r=R),
                    axis=AXX, op=ADD)

            # per-stream: r = 1/den ; y = num * r ; head-add ; transpose
            redv = red.rearrange("p (c s g) -> p c s g", s=NS, g=G)
            ys = []
            for s in range(NS):
                rs = wpool.tile([KB, NCH], F32, name="rs", tag=f"rs{s}")
                nc.vector.reciprocal_approx_fast(out=rs[:], in_=redv[:, :, s, 0])
                y_s = wpool.tile([KB, NCH, 2], F16, name="ys", tag=f"ys{s}")
                nc.vector.tensor_tensor(
                    out=y_s[:], in0=redv[:, :, s, 1:3],
                    in1=rs[:].unsqueeze(2).broadcast_to((KB, NCH, 2)), op=MULT)
                ys.append(y_s)
            for bl in range(BPC):
                yb = wpool.tile([KB, NCH * 2], F16, name="yb", tag=f"yb{bl}")
                nc.vector.tensor_tensor(
                    out=yb[:], in0=ys[2 * bl][:].rearrange("p a b -> p (a b)"),
                    in1=ys[2 * bl + 1][:].rearrange("p a b -> p (a b)"), op=ADD)
                yt = ytp.tile([NCH * 2, KB], F16, name="yt", tag=f"yt{bl}")
                nc.tensor.transpose(yt[:], yb[:], ident)
                yo = wpool.tile([NCH * 2, KB], F16, name="yo", tag=f"yo{bl}")
                if bl == 0:
                    nc.scalar.copy(yo[:], yt[:])
                else:
                    nc.vector.tensor_copy(yo[:], yt[:])
                nc.sync.dma_start(out=y_ap[bl][:], in_=yo[:])

    nc.compile()
    return nc


def _prep_inputs(x, Wq, Wk, Wv, Wo, Wboth):
    """Host-side linear prep: rank-2 factors and monomial bases, O(B*C*R)."""
    x = np.asarray(x, np.float64)
    Wq, Wk, Wv, Wo, Wboth = [np.asarray(w, np.float64)
                             for w in (Wq, Wk, Wv, Wo, Wboth)]
    pos = np.arange(C)
    pe = np.stack([np.sin(pos), np.cos(pos)], 1)           # [C,2]
    xp = x + pe[None]                                       # [B,C,2]
    A = np.einsum("hde,hfe->hdf", Wq, Wk) / np.sqrt(64)     # [H,2,2]
    M = np.stack([Wv[h] @ Wo[h] @ Wboth[h:h + 1] for h in range(H)])

    U3h, Vwh = [], []
    for h in range(H):
        Uh, sh, Vth = np.linalg.svd(A[h])
        a = xp @ (Uh * np.sqrt(sh))                         # [B,C,2]
        b = xp @ (Vth.T * np.sqrt(sh))
        u = xp @ M[h]                                       # [B,C,2]
        U = np.stack([a[..., 0] ** i * a[..., 1] ** j
                      / (math.factorial(i) * math.factorial(j))
                      for (i, j) in EXPS], -1)              # [B,C,R]
        V = np.stack([b[..., 0] ** i * b[..., 1] ** j for (i, j) in EXPS], -1)
        Vw = np.concatenate([V, V * u[..., 0:1], V * u[..., 1:2]], -1)
        U3h.append(np.tile(U, (1, 1, G)))                   # [B,C,RW]
        Vwh.append(Vw)

    # consts
    q_i = np.arange(KB)
    wts = np.zeros((KB, 544), np.float16)
    wts[:, 0:128] = (q_i[:, None] <= q_i[None, :])          # tril^T
    for ci in range(NCH):
        wts[:, 128 + 16 * ci + ci] = 1.0                    # one-hot col ci
    wts[0:NCH, 384:400] = (np.arange(NCH)[:, None]
                           < np.arange(NCH)[None, :])       # strict chunk-tril
    wts[:, 400:528] = np.eye(KB)
    wts[:, 528] = q_i                                       # partition index
    wts[:, 529:544] = np.arange(1, 16)[None, :]             # chunk index row

    in_maps = []
    for core in range(NCORES):
        vw = np.empty((KB, NCH, NS, RW), np.float16)
        u3 = np.empty((KB, NCH, NS, RW), np.float16)
        for s in range(NS):
            b_ = core * BPC + s // H
            h = s % H
            # [C, RW] -> [NCH, KB, RW] -> [KB, NCH, RW]
            vw[:, :, s] = Vwh[h][b_].reshape(NCH, KB, RW).transpose(1, 0, 2)
            u3[:, :, s] = U3h[h][b_].reshape(NCH, KB, RW).transpose(1, 0, 2)
        in_maps.append({
            "wts": wts,
            "vw": np.ascontiguousarray(vw.reshape(KB, TOT)),
            "u3": np.ascontiguousarray(u3.reshape(KB, TOT)),
        })
    return in_maps


def run(inputs, trace=False):
    from concourse.bass_utils import run_bass_kernel_spmd

    if "nc" not in _cache:
        _cache["nc"] = _build_program()
    nc = _cache["nc"]
    in_maps = _prep_inputs(**inputs)
    res = run_bass_kernel_spmd(
        nc, in_maps, core_ids=list(range(NCORES)), trace=trace)
    y = np.empty((B, C, 2), np.float32)
    for core in range(NCORES):
        for bl in range(BPC):
            yt = res.results[core][f"y{bl}"].astype(np.float32)  # [32,128]
            y[core * BPC + bl] = yt.reshape(NCH, 2, KB).transpose(
                0, 2, 1).reshape(C, 2)
    return y, res


def kernel(**inputs) -> np.ndarray:
    y, _ = run(inputs, trace=False)
    return y
